# revision 18
# baseline (speedup 1.0000x reference)
"""Trainium2 Bass kernel for the NSDE model (Euler-Maruyama scan + MLPs).

Strategy:
  - Data-parallel over batch: 16384 rows -> 8 cores x 2048 rows.
  - Only the 20 time slices of x_path that the scan actually reads are
    shipped to the device (indices computed on host from t_span).
  - Feature-major layout on chip: activations are [feature, batch] so every
    matmul uses the weight matrix directly as lhsT (out = W^T @ actT) and
    biases are per-partition scalars.
  - The per-core batch (2048) is processed as TWO independent interleaved
    streams of 1024 rows. The Euler-Maruyama scan is inherently serial, so a
    single stream leaves every engine idle most of the time; two streams
    keep the tensor/scalar/vector engines busy with the other stream while
    one waits on its dependency chain.
  - Within a stream, 64-feature tensors are "packed": partitions 0-63 hold
    features of its first 512 rows, partitions 64-127 the second 512. Drift
    layer1 uses PE row tiling (tile_position=(64,0), duplicated weights);
    diffusion layers use block-diagonal weights; drift output uses column
    tiling ((0,64) writes psum rows 64:128).
  - Bulk matmuls are bf16 (1 cycle/row). The h carry stays high precision:
    h is float32r (4-byte, ~1 cyc/row vs fp32's two half-rate passes) and
    is added into the drift-out PSUM exactly via a f32r identity matmul.
    h's consumers (drift l1 h-part, diffusion l1, identity) run f32r.
  - h' = (psum3 + dt*db3) + sigmoid*zs via one fused scalar_tensor_tensor.
  - dt and sqrt(dt) folds are done on host: dW3*dt per step, db3*dt, and
    the noise is pre-scaled zs = dW * gscale * sqrt(dt) (bf16).
  - Elementwise work is balanced across ScalarE/VectorE (both ~1 elem/cyc
    from PSUM) with the noise multiply on GpSimd.
"""

import os
from contextlib import ExitStack

import ml_dtypes
import numpy as np

import concourse.bass as bass
import concourse.mybir as mybir
import concourse.tile as tile
from concourse import bacc
from concourse.bass_utils import run_bass_kernel_spmd

F32 = mybir.dt.float32
F32R = mybir.dt.float32r
BF16 = mybir.dt.bfloat16
AF = mybir.ActivationFunctionType
ALU = mybir.AluOpType

NPBF = ml_dtypes.bfloat16

STEPS = 20
NCORES = 8
B = 16384
BC = B // NCORES  # per-core batch: 2048
SB = BC // 2  # per-stream batch: 1024
HB = SB // 2  # packed free size per stream: 512
H = 64
FX = 64
DW = 128

_CACHE = {}


def _build():
    if "nc" in _CACHE:
        return _CACHE["nc"]

    nc = bacc.Bacc("TRN2", target_bir_lowering=False, debug=False)

    def din(name, shape, dt=F32):
        return nc.dram_tensor(name, shape, dt, kind="ExternalInput")

    d_xt = din("xt", [STEPS, 128, 2, HB], BF16)  # [step, feat-packed, stream, b]
    d_zst = din("zst", [STEPS, 128, 2, HB], BF16)
    d_w1h = din("w1h", [128, DW], F32R)  # dW1[:64] duplicated (f32r: reads h)
    d_w1x = din("w1x", [128, DW], BF16)  # dW1[64:] duplicated
    d_w2 = din("w2", [DW, DW], BF16)
    d_w3s = din("w3s", [STEPS, DW, H], BF16)  # dW3 * dt_k
    d_gw1 = din("gw1", [128, DW], F32R)  # blockdiag(gW1, gW1) (f32r: reads h)
    d_gw2 = din("gw2", [DW, DW], BF16)  # blockdiag(gW2, gW2)
    d_id = din("ident", [128, DW], F32R)  # 128x128 identity (h carry)
    d_b1 = din("b1", [DW, 1])
    d_b2 = din("b2", [DW, 1])
    d_dtb3 = din("dtb3", [128, STEPS])  # dt_k * db3, packed-dup
    d_gb1 = din("gb1", [128, 1])
    d_gb2 = din("gb2", [128, 1])
    d_rw1 = din("rw1", [128, 32], F32R)  # dup
    d_rb1 = din("rb1", [32, 1])
    d_rw2 = din("rw2", [32, 2], BF16)
    d_rb2 = din("rb2", [2, 1])
    d_h0 = din("h0", [128, HB], F32R)  # zeros (f32r memset unsupported)
    d_out = nc.dram_tensor("out", [2, BC], F32, kind="ExternalOutput")

    with ExitStack() as ctx:
        tc = ctx.enter_context(tile.TileContext(nc))
        consts = ctx.enter_context(tc.tile_pool(name="consts", bufs=1))
        xzp = ctx.enter_context(tc.tile_pool(name="xzp", bufs=4))
        hp = ctx.enter_context(tc.tile_pool(name="hp", bufs=3))
        wk = ctx.enter_context(tc.tile_pool(name="wk", bufs=3))
        ppb = ctx.enter_context(tc.tile_pool(name="ppb", bufs=4, space="PSUM"))
        pps = ctx.enter_context(tc.tile_pool(name="pps", bufs=4, space="PSUM"))

        def cload(dram_ap, shape, name, dt=F32):
            t = consts.tile(shape, dt, name=name, tag=name)
            nc.sync.dma_start(t[:], dram_ap)
            return t

        w1h = cload(d_w1h[:, :], [128, DW], "w1h", F32R)
        w1x = cload(d_w1x[:, :], [128, DW], "w1x", BF16)
        w2 = cload(d_w2[:, :], [DW, DW], "w2", BF16)
        w3s = cload(
            d_w3s[:, :, :].rearrange("k p m -> p k m"), [DW, STEPS, H], "w3s", BF16
        )
        gw1 = cload(d_gw1[:, :], [128, DW], "gw1", F32R)
        gw2 = cload(d_gw2[:, :], [DW, DW], "gw2", BF16)
        idn = cload(d_id[:, :], [128, DW], "idn", F32R)
        b1 = cload(d_b1[:, :], [DW, 1], "b1")
        b2 = cload(d_b2[:, :], [DW, 1], "b2")
        dtb3 = cload(d_dtb3[:, :], [128, STEPS], "dtb3")
        gb1 = cload(d_gb1[:, :], [128, 1], "gb1")
        gb2 = cload(d_gb2[:, :], [128, 1], "gb2")
        rw1 = cload(d_rw1[:, :], [128, 32], "rw1", F32R)
        rb1 = cload(d_rb1[:, :], [32, 1], "rb1")
        rw2 = cload(d_rw2[:, :], [32, 2], "rw2", BF16)
        rb2 = cload(d_rb2[:, :], [2, 1], "rb2")

        h_cur = []
        for s in range(2):
            h0 = hp.tile([128, HB], F32R, name=f"h{s}", tag=f"h{s}")
            nc.sync.dma_start(h0[:], d_h0[:, :])
            h_cur.append(h0)

        # HAM warm-up: a dense burst of matmuls so the PE clock ungates
        # (K=8/8) before step 0 instead of ~25us into the run.
        ps_w = pps.tile([128, HB], F32, name="ps_w", tag="pps")
        for _ in range(18):
            nc.tensor.matmul(
                ps_w[:, :], idn[:, :], h_cur[0][:, :], start=True, stop=True,
                skip_group_check=True,
            )

        def step_pair(k):
            """One Euler-Maruyama step for both streams, stage-interleaved
            so consecutive PE instructions belong to alternating streams
            (each one's dependency was satisfied while the other ran)."""
            xk, zk, g1, sg, tt, z1, z2 = {}, {}, {}, {}, {}, {}, {}
            psg, pss, ps1, ps2, ps3 = {}, {}, {}, {}, {}

            xkb = xzp.tile([128, 2, HB], BF16, name="xkb", tag="xkb")
            nc.sync.dma_start(xkb[:], d_xt[k])
            zkb = xzp.tile([128, 2, HB], BF16, name="zkb", tag="zkb")
            nc.sync.dma_start(zkb[:], d_zst[k])
            for s in range(2):
                xk[s] = xkb[:, s, :]
                zk[s] = zkb[:, s, :]

            # ---- diffusion layer 1 (blockdiag weights) ----
            for s in range(2):
                psg[s] = pps.tile([128, HB], F32, name=f"psg{s}", tag="pps")
                nc.tensor.matmul(
                    psg[s][:, :], gw1[:, :], h_cur[s][:, :], start=True, stop=True
                )
            for s in range(2):
                g1[s] = wk.tile([128, HB], BF16, name=f"g1{s}", tag=f"g1{s}")
                if s == 0:
                    nc.scalar.activation(g1[s][:], psg[s][:], AF.Relu, bias=gb1[:])
                else:
                    nc.vector.tensor_scalar(
                        g1[s][:], psg[s][:], gb1[:], 0.0, ALU.add, ALU.max
                    )

            # ---- drift layer 1 matmuls (row-tiled) ----
            for s in range(2):
                ps1[s] = [
                    ppb.tile([128, HB], F32, name=f"ps1{s}{j}", tag="ppb")
                    for j in range(2)
                ]
                for j, tp in ((0, None), (1, (64, 0))):
                    lo, hi = 64 * j, 64 * (j + 1)
                    nc.tensor.matmul(
                        ps1[s][j][:, :], w1h[lo:hi, :], h_cur[s][lo:hi, :],
                        start=True, stop=False, tile_position=tp,
                    )
                    nc.tensor.matmul(
                        ps1[s][j][:, :], w1x[lo:hi, :], xk[s][lo:hi, :],
                        start=False, stop=True, tile_position=tp,
                    )

            # ---- diffusion layer 2 + sigmoid + noise ----
            for s in range(2):
                pss[s] = pps.tile([128, HB], F32, name=f"pss{s}", tag="pps")
                nc.tensor.matmul(
                    pss[s][:, :], gw2[:, :], g1[s][:, :], start=True, stop=True
                )
            for s in range(2):
                sg[s] = wk.tile([128, HB], BF16, name=f"sg{s}", tag=f"sg{s}")
                nc.scalar.activation(sg[s][:], pss[s][:], AF.Sigmoid, bias=gb2[:])
                tt[s] = wk.tile([128, HB], BF16, name=f"tt{s}", tag=f"tt{s}")
                nc.gpsimd.tensor_mul(tt[s][:], sg[s][:], zk[s][:])

            # ---- drift layer 1 relu (split ACT/DVE) ----
            for s in range(2):
                z1[s] = wk.tile([128, SB], BF16, name=f"z1{s}", tag=f"z1{s}")
                nc.scalar.activation(
                    z1[s][:, 0:HB], ps1[s][0][:], AF.Relu, bias=b1[:]
                )
                nc.vector.tensor_scalar(
                    z1[s][:, HB:], ps1[s][1][:], b1[:], 0.0, ALU.add, ALU.max
                )

            # ---- drift layer 2 ----
            for s in range(2):
                ps2[s] = [
                    ppb.tile([128, HB], F32, name=f"ps2{s}{j}", tag="ppb")
                    for j in range(2)
                ]
                for j in range(2):
                    nc.tensor.matmul(
                        ps2[s][j][:, :], w2[:, :], z1[s][:, j * HB : (j + 1) * HB],
                        start=True, stop=True,
                    )
            for s in range(2):
                z2[s] = wk.tile([128, SB], BF16, name=f"z2{s}", tag=f"z2{s}")
                nc.scalar.activation(
                    z2[s][:, 0:HB], ps2[s][0][:], AF.Relu, bias=b2[:]
                )
                nc.vector.tensor_scalar(
                    z2[s][:, HB:], ps2[s][1][:], b2[:], 0.0, ALU.add, ALU.max
                )

            # ---- drift out + h carry: ps3 = h + dt*(z2 @ dW3) ----
            for s in range(2):
                ps3[s] = pps.tile([128, HB], F32, name=f"ps3{s}", tag="pps")
                nc.tensor.matmul(
                    ps3[s][:, :], idn[:, :], h_cur[s][:, :],
                    start=True, stop=False, skip_group_check=True,
                )
            for s in range(2):
                nc.tensor.matmul(
                    ps3[s][0:64, :], w3s[:, k, :], z2[s][:, 0:HB],
                    start=False, stop=False, skip_group_check=True,
                )
                nc.tensor.matmul(
                    ps3[s][64:128, :], w3s[:, k, :], z2[s][:, HB:],
                    start=False, stop=True, tile_position=(0, 64),
                    skip_group_check=True,
                )
            # h' = (ps3 + dt*db3) + sigmoid*zs
            for s in range(2):
                h_new = hp.tile([128, HB], F32R, name=f"h{s}", tag=f"h{s}")
                nc.vector.scalar_tensor_tensor(
                    h_new[:], ps3[s][:], dtb3[:, k : k + 1], tt[s][:],
                    ALU.add, ALU.add,
                )
                h_cur[s] = h_new

        for k in range(STEPS):
            step_pair(k)

        # ---- readout: out = relu(h @ rW1 + rb1) @ rW2 + rb2 ----
        # per stream, straight from the packed h (row-tiled K=64 halves)
        r1 = wk.tile([32, BC], BF16, name="r1", tag="r1")
        for s in range(2):
            for j, tp in ((0, None), (1, (64, 0))):
                lo, hi = 64 * j, 64 * (j + 1)
                sl = slice(s * SB + j * HB, s * SB + (j + 1) * HB)
                psr = pps.tile([128, HB], F32, name="psr", tag="pps")
                nc.tensor.matmul(
                    psr[0:32, :], rw1[lo:hi, :], h_cur[s][lo:hi, :],
                    start=True, stop=True, tile_position=tp,
                )
                nc.scalar.activation(r1[:, sl], psr[0:32, :], AF.Relu, bias=rb1[:])

        osb = wk.tile([2, BC], F32, name="osb", tag="osb")
        for q in range(4):
            sl = slice(q * HB, (q + 1) * HB)
            pso = pps.tile([128, HB], F32, name="pso", tag="pps")
            nc.tensor.matmul(
                pso[0:2, :], rw2[:, :], r1[:, sl], start=True, stop=True
            )
            nc.scalar.activation(osb[:, sl], pso[0:2, :], AF.Identity, bias=rb2[:])
        nc.sync.dma_start(d_out[:, :], osb[:])

    nc.compile()
    _CACHE["nc"] = nc
    return nc


def _dup(a, dt=NPBF):
    return np.ascontiguousarray(np.concatenate([a, a], axis=0).astype(dt))


def _blkdiag(a, dt=NPBF):
    n, m = a.shape
    out = np.zeros((2 * n, 2 * m), np.float32)
    out[:n, :m] = a
    out[n:, m:] = a
    return np.ascontiguousarray(out.astype(dt))


def _prep_in_maps(inputs):
    xp = np.asarray(inputs["x_path"], dtype=np.float32)
    t_span = np.asarray(inputs["t_span"], dtype=np.float32)
    dw = np.asarray(inputs["dW"], dtype=np.float32)

    Tm1 = np.int32(xp.shape[1] - 1)
    t_max = t_span[-1]
    idx = np.clip(
        (t_span[:-1] / t_max * np.float32(Tm1)).astype(np.int32), 0, Tm1
    )
    dts = (t_span[1:] - t_span[:-1]).astype(np.float32)
    sq = np.sqrt(dts).astype(np.float32)

    gscale = np.asarray(inputs["gscale"], dtype=np.float32)
    w1 = np.asarray(inputs["dW1"], dtype=np.float32)
    w2 = np.asarray(inputs["dW2"], dtype=np.float32)
    w3 = np.asarray(inputs["dW3"], dtype=np.float32)
    db1 = np.asarray(inputs["db1"], dtype=np.float32)
    db2 = np.asarray(inputs["db2"], dtype=np.float32)
    db3 = np.asarray(inputs["db3"], dtype=np.float32)
    gw1 = np.asarray(inputs["gW1"], dtype=np.float32)
    gw2 = np.asarray(inputs["gW2"], dtype=np.float32)
    gb1 = np.asarray(inputs["gb1"], dtype=np.float32)
    gb2 = np.asarray(inputs["gb2"], dtype=np.float32)
    rw1 = np.asarray(inputs["rW1"], dtype=np.float32)
    rb1 = np.asarray(inputs["rb1"], dtype=np.float32)
    rw2 = np.asarray(inputs["rW2"], dtype=np.float32)
    rb2 = np.asarray(inputs["rb2"], dtype=np.float32)

    w3s = w3[None, :, :] * dts[:, None, None]  # [STEPS, DW, H]

    common = {
        "w1h": _dup(w1[:H], np.float32),
        "w1x": _dup(w1[H:]),
        "w2": np.ascontiguousarray(w2.astype(NPBF)),
        "w3s": np.ascontiguousarray(w3s.astype(NPBF)),
        "gw1": _blkdiag(gw1, np.float32),
        "gw2": _blkdiag(gw2),
        "ident": np.eye(DW, dtype=np.float32),
        "b1": np.ascontiguousarray(db1.reshape(DW, 1)),
        "b2": np.ascontiguousarray(db2.reshape(DW, 1)),
        "dtb3": _dup((dts[:, None] * db3[None, :]).T, np.float32),  # [128, STEPS]
        "gb1": _dup(gb1.reshape(H, 1), np.float32),
        "gb2": _dup(gb2.reshape(H, 1), np.float32),
        "rw1": _dup(rw1, np.float32),
        "rb1": np.ascontiguousarray(rb1.reshape(32, 1)),
        "rw2": np.ascontiguousarray(rw2.astype(NPBF)),
        "rb2": np.ascontiguousarray(rb2.reshape(2, 1)),
        "h0": np.zeros((128, HB), np.float32),
    }

    xg = xp[:, idx, :]  # [B, STEPS, F]
    zsc = gscale[None, :] * sq[:, None]  # [STEPS, F]

    in_maps = []
    for c in range(NCORES):
        rows = slice(c * BC, (c + 1) * BC)
        # (stream, half, b', k, f) -> (k, stream, half, f, b')
        xt = np.ascontiguousarray(
            xg[rows]
            .reshape(2, 2, HB, STEPS, FX)
            .transpose(3, 1, 4, 0, 2)
            .reshape(STEPS, 128, 2, HB)
            .astype(NPBF)
        )
        zc = dw[:, rows, :] * zsc[:, None, :]  # [STEPS, BC, H]
        zst = np.ascontiguousarray(
            zc.reshape(STEPS, 2, 2, HB, H)
            .transpose(0, 2, 4, 1, 3)
            .reshape(STEPS, 128, 2, HB)
            .astype(NPBF)
        )
        m = dict(common)
        m["xt"] = xt
        m["zst"] = zst
        in_maps.append(m)
    return in_maps


def kernel(**inputs):
    nc = _build()
    in_maps = _prep_in_maps(inputs)
    run_kwargs = dict(_CACHE.get("run_kwargs", {}))
    res = run_bass_kernel_spmd(nc, in_maps, list(range(NCORES)), **run_kwargs)
    _CACHE["last_results"] = res
    mu = np.concatenate([res.results[c]["out"][0] for c in range(NCORES)])
    ls = np.concatenate([res.results[c]["out"][1] for c in range(NCORES)])
    return mu, ls
